# revision 3
# baseline (speedup 1.0000x reference)
"""BloomAttention (B=1, S=2048, HID=4096, NH=32) on 8 Trainium2 NeuronCores.

Strategy (tensor-parallel over heads, as the module does):
  - Each core owns 4 heads. w_qkv/b_qkv column-sharded (per-head q/k/v rows),
    INV_NORM folded into the q slice on host; weights shipped transposed+bf16,
    activations shipped bf16 (compute dtype).
  - On-device: hiddenT tiles via xbar DMA-transpose; QKV matmul produces
    qT/kT [d, s] per head directly, V staged to DRAM and transpose-loaded
    back as natural [s, d] for the PV matmul.
  - Attention in transposed-scores layout: scoresT[sk, sq] = kT.T @ qT.
    ALiBi bias + per-query shift + causal mask are all applied in ONE vector
    op per tile: ps += slope_h * D[a,b] where D = (sk - sq) on causal-valid
    entries and -4e9 on masked ones. D depends only on the 128-aligned tile
    offset (19 distinct tiles, SBUF-resident). The shift (-slope*sq) is
    exact: softmax is shift-invariant per query, and the diagonal term
    bounds exp() so no max-reduce is needed. exp on ACT; P@V and the
    softmax denominator are matmuls over the sk partitions (ones column),
    software-pipelined behind the score matmuls; normalization uses a
    ones-row broadcast matmul + reciprocal.
  - AllToAll swaps head-shards for sequence-shards of the context, then each
    core computes its 256 output rows against the full (transposed, bf16)
    w_dense. Host just concatenates the 8 row-shards.

Note: assumes the alibi input is the standard Bloom form alibi[h, j] =
slope_h * j (slope read from alibi[:, 1]); the reference's setup_inputs
builds exactly that.
"""

import math
import os
import sys
import types
from contextlib import ExitStack

import numpy as np
import ml_dtypes

B, S, HID, NH, HD = 1, 2048, 4096, 32, 128
NCORES = 8
NH_LOC = NH // NCORES            # 4 heads per core
FQKV = NH_LOC * 3 * HD           # 1536 qkv features per core
SROW = S // NCORES               # 256 output rows per core
INV_NORM = 1.0 / math.sqrt(HD)
KT = HID // HD                   # 32 k tiles
KC = 12                          # k tiles cached in SBUF (rest streamed)
KS = KT - KC                     # streamed k tiles
NR = 19                          # distinct (sk-sq)/128 tile offsets: -15..3

_CACHE = {}


def _ensure_axon_hooks():
    try:
        import antenv  # noqa: F401

        extra = "/opt/trn_rl_repo/antenv"
        if os.path.isdir(extra) and extra not in antenv.__path__:
            antenv.__path__.append(extra)
        import antenv.axon_hooks  # noqa: F401
    except Exception:
        if "antenv.axon_hooks" in sys.modules:
            return
        # Functional stand-in: the axon boot code (trn_boot.py) stores the
        # NTFF profiling hook here at jax init; bass_utils reads it back.
        m = types.ModuleType("antenv.axon_hooks")
        m._hook = None

        def _set(h, _m=m):
            _m._hook = h

        m.get_axon_ntff_profile_hook = lambda _m=m: _m._hook
        m.set_axon_ntff_profile_hook = _set
        sys.modules["antenv.axon_hooks"] = m
        try:
            from trn_agent_boot.trn_boot import _ntff_profile_via_ctypes

            so = "/opt/axon/libaxon_pjrt.so"
            if os.path.isfile(so):
                hook = _ntff_profile_via_ctypes(so)
                if hook is not None:
                    m._hook = hook
        except Exception:
            pass


_ensure_axon_hooks()


def _kt_order():
    cached = list(range(KC))
    streamed = list(range(KC, KT))
    order = []
    for i in range(max(len(cached), len(streamed))):
        if i < len(cached):
            order.append(cached[i])
        if i < len(streamed):
            order.append(streamed[i])
    return order


def _build_nc():
    import concourse.bass as bass  # noqa: F401
    import concourse.mybir as mybir
    from concourse import bacc, tile

    BF = mybir.dt.bfloat16
    F32 = mybir.dt.float32
    Alu = mybir.AluOpType
    Act = mybir.ActivationFunctionType

    nc = bacc.Bacc(None, target_bir_lowering=False, num_devices=NCORES)
    with tile.TileContext(nc) as tc, ExitStack() as ctx:
        dram = ctx.enter_context(tc.tile_pool(name="dram", bufs=1, space="DRAM"))

        def din(name, shape, dt):
            return dram.tile(shape, dt, kind="ExternalInput", name=name,
                             uniquify=False)

        hidden = din("hidden", [S, HID], BF)
        wqcd = din("wqc", [HD, KC, FQKV], BF)
        wstrd = din("wstr", [2, HD, KS, 768], BF)
        bqkv = din("bqkv", [HD, NH_LOC * 3], F32)
        dmatd = din("dmat", [HD, NR * 512], F32)
        slopesd = din("slopes", [HD, NH_LOC], F32)
        wdr = din("wdr", [8, HD, KT, 512], BF)
        bdense = din("bdense", [1, HID], F32)
        out = dram.tile([SROW, HID], F32, kind="ExternalOutput", name="out",
                        uniquify=False)
        a2a_in = [dram.tile([NCORES, 2, HD, SROW], BF, name=f"a2a_in{p}")
                  for p in range(2)]
        a2a_out = [dram.tile([NCORES, 2, HD, SROW], BF, name=f"a2a_out{p}")
                   for p in range(2)]
        vdram = dram.tile([NH_LOC, HD, S], BF, name="vdram")

        # ---------- persistent SBUF ----------
        const = ctx.enter_context(tc.tile_pool(name="const", bufs=1))
        sb_bqkv = const.tile([HD, NH_LOC * 3], F32)
        nc.sync.dma_start(out=sb_bqkv[:], in_=bqkv[:])
        sb_slopes = const.tile([HD, NH_LOC], F32)
        nc.sync.dma_start(out=sb_slopes[:], in_=slopesd[:])
        ones_col = const.tile([HD, 1], BF)
        nc.vector.memset(ones_col[:], 1.0)
        ones_row = const.tile([1, HD], F32)
        nc.vector.memset(ones_row[:], 1.0)

        persist = ctx.enter_context(tc.tile_pool(name="persist", bufs=1))
        qT = [persist.tile([HD, S], BF, name=f"qT{h}") for h in range(NH_LOC)]
        kTt = [persist.tile([HD, S], BF, name=f"kT{h}") for h in range(NH_LOC)]
        vnat = [persist.tile([HD, S], BF, name=f"vn{h}")
                for h in range(NH_LOC)]

        # ---------- phase 1: QKV ----------
        FG = [list(range(0, 6)), list(range(6, 12))]
        KORD = _kt_order()
        with (
            tc.tile_pool(name="wqc", bufs=1) as wqc_pool,
            tc.tile_pool(name="wstream", bufs=2) as ws_pool,
            tc.tile_pool(name="hT", bufs=2) as hT_pool,
            tc.tile_pool(name="vstg", bufs=3) as vstg_pool,
            tc.tile_pool(name="qkv_ps", bufs=1, space="PSUM") as qkv_ps,
        ):
            wq_c = wqc_pool.tile([HD, KC, FQKV], BF)
            nc.sync.dma_start(out=wq_c[:], in_=wqcd[:])

            for sq in range(4):  # s-quarters of 512
                s0 = sq * 512
                hT_q = hT_pool.tile([HD, KT, 512], BF, name="hT_q")
                for kt in KORD:
                    nc.scalar.dma_start(
                        out=hT_q[:, kt, :],
                        in_=hidden[s0:s0 + 512, kt * HD:(kt + 1) * HD],
                        transpose=True)
                for fg in FG:
                    nf = len(fg)
                    f0 = fg[0] * HD
                    psl = [qkv_ps.tile([HD, 512], F32, name=f"qkvps{i}",
                                       bufs=1) for i in range(nf)]
                    # two big prefetch DMAs for the streamed half of K
                    fgi = fg[0] // 6
                    half_n = KS // 2
                    wsts = []
                    for half in range(2):
                        k0 = half * half_n
                        wst = ws_pool.tile([HD, half_n, 6 * HD], BF,
                                           name="ws")
                        nc.sync.dma_start(
                            out=wst[:],
                            in_=wstrd[fgi, :, k0:k0 + half_n, :])
                        wsts.append(wst)
                    for ki, kt in enumerate(KORD):
                        if kt < KC:
                            wsl = wq_c[:, kt, f0:f0 + nf * HD]
                        else:
                            wsl = wsts[(kt - KC) // half_n][
                                :, (kt - KC) % half_n, :]
                        for i in range(nf):
                            nc.tensor.matmul(
                                psl[i][:],
                                wsl[:, i * HD:(i + 1) * HD],
                                hT_q[:, kt, :],
                                start=(ki == 0), stop=(ki == KT - 1))
                    for i, ft in enumerate(fg):
                        h, j = divmod(ft, 3)
                        if j < 2:
                            dest = (qT, kTt)[j][h][:, s0:s0 + 512]
                            nc.scalar.activation(
                                dest, psl[i][:], Act.Identity,
                                bias=sb_bqkv[:, ft:ft + 1])
                        else:
                            vs = vstg_pool.tile([HD, 512], BF, name="vs")
                            nc.scalar.activation(
                                vs[:], psl[i][:], Act.Identity,
                                bias=sb_bqkv[:, ft:ft + 1])
                            nc.sync.dma_start(
                                out=vdram[h, :, s0:s0 + 512], in_=vs[:])
                            for t4 in range(4):
                                sk0 = s0 + t4 * HD
                                nc.scalar.dma_start(
                                    out=vnat[h][:, sk0:sk0 + HD],
                                    in_=vdram[h, :, sk0:sk0 + HD],
                                    transpose=True)

        # ---------- phase 2: attention ----------
        with (
            tc.tile_pool(name="attn_sb", bufs=1) as attn_sb,
            tc.tile_pool(name="expp", bufs=4) as expp,
            tc.tile_pool(name="bcp", bufs=2) as bcp,
            tc.tile_pool(name="attn_ps", bufs=1, space="PSUM") as attn_ps,
            tc.tile_pool(name="sc_ps", bufs=4, space="PSUM") as sc_ps,
        ):
            dmat = attn_sb.tile([HD, NR * 512], F32)
            nc.sync.dma_start(out=dmat[:], in_=dmatd[:])
            ctxT = [attn_sb.tile([HD, S], BF, name=f"cx{h}")
                    for h in range(NH_LOC)]

            for h in range(NH_LOC):
                slope = sb_slopes[:, h:h + 1]
                for sqb in range(4):
                    q0 = sqb * 512
                    nsk = 4 * (sqb + 1)
                    ps_ctx = attn_ps.tile([HD, 512], F32, name="ps_ctx", bufs=2)
                    ps_sum = attn_ps.tile([1, 512], F32, name="ps_sum", bufs=1)
                    exs = {}

                    def flush(skt, first, last):
                        ex = exs.pop(skt)
                        nc.tensor.matmul(
                            ps_ctx[:], vnat[h][:, skt * HD:(skt + 1) * HD],
                            ex[:], start=first, stop=last)
                        nc.tensor.matmul(
                            ps_sum[:], ones_col[:], ex[:],
                            start=first, stop=last)

                    for skt in range(nsk):
                        ri = skt - 4 * sqb + 15  # (sk0-q0)/128 + 15
                        ps = sc_ps.tile([HD, 512], F32, name="ps_sc")
                        nc.tensor.matmul(
                            ps[:], kTt[h][:, skt * HD:(skt + 1) * HD],
                            qT[h][:, q0:q0 + 512], start=True, stop=True)
                        nc.vector.scalar_tensor_tensor(
                            ps[:], dmat[:, ri * 512:(ri + 1) * 512], slope,
                            ps[:], Alu.mult, Alu.add)
                        ex = expp.tile([HD, 512], BF, name="ex")
                        nc.scalar.activation(ex[:], ps[:], Act.Exp)
                        exs[skt] = ex
                        if skt >= 2:
                            flush(skt - 2, skt - 2 == 0, False)
                    for skt in (nsk - 2, nsk - 1):
                        flush(skt, skt == 0, skt == nsk - 1)

                    ps_bc = attn_ps.tile([HD, 512], F32, name="ps_bc", bufs=1)
                    sum_sb = bcp.tile([1, 512], F32, name="sum_sb")
                    nc.scalar.copy(sum_sb[:], ps_sum[:])
                    nc.tensor.matmul(ps_bc[:], ones_row[:], sum_sb[:],
                                     start=True, stop=True)
                    rec_bc = bcp.tile([HD, 512], F32, name="rec_bc")
                    nc.vector.reciprocal(rec_bc[:], ps_bc[:])
                    nc.vector.tensor_tensor(
                        ctxT[h][:, q0:q0 + 512], ps_ctx[:], rec_bc[:],
                        Alu.mult)
                    for j in (2 * sqb, 2 * sqb + 1):
                        nc.sync.dma_start(
                            out=a2a_in[h // 2][j, h % 2],
                            in_=ctxT[h][:, j * SROW:(j + 1) * SROW])

            # ---------- phase 3: all-to-all ----------
            for p in range(2):
                nc.gpsimd.collective_compute(
                    "AllToAll", Alu.bypass,
                    replica_groups=[list(range(NCORES))],
                    ins=[a2a_in[p][:]], outs=[a2a_out[p][:]],
                )

        # ---------- phase 4: dense ----------
        with (
            tc.tile_pool(name="dns_sb", bufs=1) as dns_sb,
            tc.tile_pool(name="wd_pool", bufs=2) as wd_pool,
            tc.tile_pool(name="osb_pool", bufs=3) as osb_pool,
            tc.tile_pool(name="dns_ps", bufs=3, space="PSUM") as dns_ps,
        ):
            sb_bd = dns_sb.tile([1, HID], F32)
            nc.sync.dma_start(out=sb_bd[:], in_=bdense[:])
            crecv = dns_sb.tile([HD, KT, SROW], BF)
            for i in range(NCORES):
                for p in range(2):
                    nc.sync.dma_start(
                        out=crecv[:, i * NH_LOC + p * 2:
                                  i * NH_LOC + p * 2 + 2, :],
                        in_=a2a_out[p][i].rearrange("l p s -> p l s"))
            for ot in range(8):
                o0 = ot * 512
                wd = wd_pool.tile([HD, KT, 512], BF, name="wd")
                nc.sync.dma_start(out=wd[:], in_=wdr[ot])
                for st in range(2):
                    psd = dns_ps.tile([HD, 512], F32, name="psd")
                    for ft in range(KT):
                        nc.tensor.matmul(
                            psd[:], crecv[:, ft, st * HD:(st + 1) * HD],
                            wd[:, ft, :], start=(ft == 0), stop=False)
                    nc.tensor.matmul(
                        psd[:], ones_row[:], sb_bd[:, o0:o0 + 512],
                        start=False, stop=True)
                    osb = osb_pool.tile([HD, 512], F32, name="osb")
                    nc.scalar.copy(osb[:], psd[:])
                    nc.sync.dma_start(
                        out=out[st * HD:(st + 1) * HD, o0:o0 + 512],
                        in_=osb[:])
    nc.compile()
    return nc


def _prep_shards(hidden_states, alibi, w_qkv, b_qkv, w_dense, b_dense):
    bf16 = ml_dtypes.bfloat16
    hidden = np.ascontiguousarray(
        np.asarray(hidden_states, dtype=np.float32).reshape(S, HID)
    ).astype(bf16)
    al = np.asarray(alibi, dtype=np.float32).reshape(NH, S)
    w = np.asarray(w_qkv, dtype=np.float32)
    b = np.asarray(b_qkv, dtype=np.float32)
    wd = np.asarray(w_dense, dtype=np.float32)
    bd = np.asarray(b_dense, dtype=np.float32)

    # fold INV_NORM into the q projections
    scale = np.ones(3 * HID, np.float32)
    for h in range(NH):
        scale[h * 3 * HD:(h * 3 * HD) + HD] = INV_NORM
    wT = np.ascontiguousarray((w * scale[:, None]).T)      # [HID, 3*HID]
    bs = b * scale
    # dense weight, transposed then tiled [8 ot][32 ft][128 f][512 o]
    wdT = np.ascontiguousarray(wd.T).astype(bf16)          # [HID(f), HID(o)]
    wdr = np.ascontiguousarray(
        wdT.reshape(KT, HD, 8, 512).transpose(2, 1, 0, 3))
    bdr = np.ascontiguousarray(bd.reshape(1, HID))

    # D tiles: for r-offset index ri (0..18), D[a, b] = (ri-15)*128 + a - b
    # where causal-valid (<= 0), else -4e9
    a = np.arange(HD)[:, None]
    bq = np.arange(512)[None, :]
    dm = []
    for ri in range(NR):
        dv = ((ri - 15) * HD + a - bq).astype(np.float32)
        dm.append(np.where(dv <= 0, dv, np.float32(-4.0e9)))
    dmat = np.concatenate(dm, axis=1)                       # [128, 19*512]

    in_maps = []
    for c in range(NCORES):
        f0 = c * FQKV
        heads = list(range(c * NH_LOC, (c + 1) * NH_LOC))
        alc = al[heads]                                     # [4, S]
        slopes = np.repeat(alc[:, 1:2].T, HD, axis=0)       # [128, 4]
        wTc = wT[:, f0:f0 + FQKV].astype(bf16)              # [HID, 1536]
        # cached half: [128, KC, 1536] partition-contiguous
        wqc = np.ascontiguousarray(
            wTc[:KC * HD].reshape(KC, HD, FQKV).transpose(1, 0, 2))
        # streamed half, pre-split by fg column group: [2, 128, KS, 768]
        wstr = np.ascontiguousarray(
            wTc[KC * HD:].reshape(KS, HD, 2, 768).transpose(2, 1, 0, 3))
        in_maps.append({
            "hidden": hidden,
            "wqc": wqc,
            "wstr": wstr,
            "bqkv": np.ascontiguousarray(
                bs[f0:f0 + FQKV].reshape(NH_LOC * 3, HD).T),
            "dmat": dmat,
            "slopes": np.ascontiguousarray(slopes.astype(np.float32)),
            "wdr": wdr,
            "bdense": bdr,
        })
    return in_maps


def kernel(hidden_states, alibi, w_qkv, b_qkv, w_dense, b_dense):
    _ensure_axon_hooks()
    from concourse import bass_utils

    if "nc" not in _CACHE:
        _CACHE["nc"] = _build_nc()
    nc = _CACHE["nc"]
    in_maps = _prep_shards(hidden_states, alibi, w_qkv, b_qkv,
                           w_dense, b_dense)
    trace = bool(os.environ.get("BLOOM_TRACE"))
    res = bass_utils.run_bass_kernel_spmd(
        nc, in_maps, core_ids=list(range(NCORES)), trace=trace)
    kernel._last_results = res
    kernel._last_exec_ns = res.exec_time_ns
    outp = np.concatenate([res.results[c]["out"] for c in range(NCORES)],
                          axis=0)
    return outp.reshape(B, S, HID).astype(np.float32)



# revision 5
# speedup vs baseline: 1.0261x; 1.0261x over previous
"""BloomAttention (B=1, S=2048, HID=4096, NH=32) on 8 Trainium2 NeuronCores.

Strategy (tensor-parallel over heads):
  - Each core owns 4 heads. w_qkv/b_qkv column-sharded; INV_NORM folded into
    the q columns on host; weights shipped transposed+bf16; hidden shipped
    PRE-TRANSPOSED (hiddenT [HID, S]) in bf16 so no on-device DMA-transpose
    is needed.
  - QKV: qT/kT [d, s] come from w.T @ hT matmuls; V is produced directly in
    NATURAL [s, d] layout by swapping the matmul operands (lhsT = hT tile,
    rhs = V weight columns), so no transpose / DRAM round-trip for V.
    V bias is folded in as a K=1 ones-row matmul at accumulation start.
  - Attention in transposed-scores layout: scoresT[sk, sq] = kT.T @ qT.
    The ALiBi bias slope*(sk-sq) (with the exact per-query shift) is applied
    as: (a) a K=1 rank-1 matmul adding slope*(-sq) (per-query-constant
    rounding cancels in softmax), (b) a per-partition bias slope*(tile_off +
    sk_within_tile) fused into the exp activation on ACT (free), and (c) a
    single shared [128,128] additive -1e9 mask strip on the causal diagonal.
    Diagonal score tiles are column-narrowed (fully-masked columns skipped).
    exp on ACT; P@V and the softmax denominator are matmuls over the sk
    partitions; normalization via ones-row broadcast matmul +
    reciprocal_approx_fast.
  - AllToAll (split in two, per head-pair) swaps head-shards for
    sequence-shards; dense is split into two k-half passes, one per
    AllToAll, so pass 0 overlaps attention of heads 2,3 and the second
    collective. Pass 1 accumulates into the DRAM output via CCE accum-DMA.

Note: assumes the alibi input is the standard Bloom form alibi[h, j] =
slope_h * j (slope read from alibi[:, 1]); the reference's setup_inputs
builds exactly that.
"""

import math
import os
import sys
import types
from contextlib import ExitStack

import numpy as np
import ml_dtypes

B, S, HID, NH, HD = 1, 2048, 4096, 32, 128
NCORES = 8
NH_LOC = NH // NCORES            # 4 heads per core
SROW = S // NCORES               # 256 output rows per core
INV_NORM = 1.0 / math.sqrt(HD)
KT = HID // HD                   # 32 contraction tiles
KC = 8                           # k tiles cached in SBUF (rest streamed)
KS = KT - KC                     # streamed k tiles (24)
NR = 19                          # distinct (sk-sq)/128 tile offsets

_CACHE = {}


def _ensure_axon_hooks():
    try:
        import antenv  # noqa: F401

        extra = "/opt/trn_rl_repo/antenv"
        if os.path.isdir(extra) and extra not in antenv.__path__:
            antenv.__path__.append(extra)
        import antenv.axon_hooks  # noqa: F401
    except Exception:
        if "antenv.axon_hooks" in sys.modules:
            return
        # Functional stand-in: the axon boot code (trn_boot.py) stores the
        # NTFF profiling hook here at jax init; bass_utils reads it back.
        m = types.ModuleType("antenv.axon_hooks")
        m._hook = None

        def _set(h, _m=m):
            _m._hook = h

        m.get_axon_ntff_profile_hook = lambda _m=m: _m._hook
        m.set_axon_ntff_profile_hook = _set
        sys.modules["antenv.axon_hooks"] = m
        try:
            from trn_agent_boot.trn_boot import _ntff_profile_via_ctypes

            so = "/opt/axon/libaxon_pjrt.so"
            if os.path.isfile(so):
                hook = _ntff_profile_via_ctypes(so)
                if hook is not None:
                    m._hook = hook
        except Exception:
            pass


_ensure_axon_hooks()


def _build_nc():
    import concourse.bass as bass  # noqa: F401
    import concourse.mybir as mybir
    from concourse import bacc, tile

    BF = mybir.dt.bfloat16
    F32 = mybir.dt.float32
    Alu = mybir.AluOpType
    Act = mybir.ActivationFunctionType

    nc = bacc.Bacc(None, target_bir_lowering=False, num_devices=NCORES)
    with tile.TileContext(nc) as tc, ExitStack() as ctx:
        dram = ctx.enter_context(tc.tile_pool(name="dram", bufs=1, space="DRAM"))

        def din(name, shape, dt):
            return dram.tile(shape, dt, kind="ExternalInput", name=name,
                             uniquify=False)

        hiddenT = din("hiddenT", [HID, S], BF)
        # [p, kt, 1536]: cols = [fg0 qk 512 | fg1 qk 512 | v 512]
        wqall = din("wqall", [HD, KT, 1536], BF)
        bqk = din("bqk", [HD, 8], F32)          # per-feature q/k bias columns
        vbias = din("vbias", [1, 512], BF)      # v bias row [4h x 128d]
        biasca = din("biasca", [HD, NH_LOC * NR], F32)  # slope*(off+a)
        negb = din("negbr", [1, 512], BF)       # -(0..511)
        slrow = din("slrow", [1, NH_LOC * HD], BF)  # slope_h replicated
        maskst = din("maskst", [HD, HD], F32)   # 0 / -1e9 strip
        wdr = din("wdr", [2, 8, HD, 16, 512], BF)
        bdense = din("bdense", [1, HID], F32)
        out = dram.tile([SROW, HID], F32, kind="ExternalOutput", name="out",
                        uniquify=False)
        a2a_in = [dram.tile([NCORES, 2, HD, SROW], BF, name=f"a2a_in{p}")
                  for p in range(2)]
        a2a_out = [dram.tile([NCORES, 2, HD, SROW], BF, name=f"a2a_out{p}")
                   for p in range(2)]

        # ---------- persistent SBUF ----------
        const = ctx.enter_context(tc.tile_pool(name="const", bufs=1))
        sb_bqk = const.tile([HD, 8], F32)
        nc.sync.dma_start(out=sb_bqk[:], in_=bqk[:])
        sb_vbias = const.tile([1, 512], BF)
        nc.sync.dma_start(out=sb_vbias[:], in_=vbias[:])
        sb_bca = const.tile([HD, NH_LOC * NR], F32)
        nc.sync.dma_start(out=sb_bca[:], in_=biasca[:])
        sb_negb = const.tile([1, 512], BF)
        nc.sync.dma_start(out=sb_negb[:], in_=negb[:])
        sb_slrow = const.tile([1, NH_LOC * HD], BF)
        nc.sync.dma_start(out=sb_slrow[:], in_=slrow[:])
        sb_mask = const.tile([HD, HD], F32)
        nc.sync.dma_start(out=sb_mask[:], in_=maskst[:])
        ones_col = const.tile([HD, 1], BF)
        nc.vector.memset(ones_col[:], 1.0)
        ones_row = const.tile([1, HD], F32)
        nc.vector.memset(ones_row[:], 1.0)
        ones_rowb = const.tile([1, HD], BF)
        nc.vector.memset(ones_rowb[:], 1.0)

        persist = ctx.enter_context(tc.tile_pool(name="persist", bufs=1))
        qT = [persist.tile([HD, S], BF, name=f"qT{h}") for h in range(NH_LOC)]
        kTt = [persist.tile([HD, S], BF, name=f"kT{h}") for h in range(NH_LOC)]
        vnat = persist.tile([HD, NH_LOC, S], BF, name="vnat")

        # ---------- phase 1: QKV ----------
        with (
            tc.tile_pool(name="wqc", bufs=1) as wqc_pool,
            tc.tile_pool(name="wstream", bufs=2) as ws_pool,
            tc.tile_pool(name="hT", bufs=2) as hT_pool,
            tc.tile_pool(name="qkv_ps", bufs=1, space="PSUM") as qkv_ps,
        ):
            wq_c = wqc_pool.tile([HD, KC, 1536], BF)
            nc.sync.dma_start(out=wq_c[:], in_=wqall[:, :KC, :])

            for sq in range(4):  # s-quarters of 512
                s0 = sq * 512
                hT_q = hT_pool.tile([HD, KT, 512], BF, name="hT_q")
                hsl = hiddenT[:, s0:s0 + 512].rearrange(
                    "(k p) s -> p k s", p=HD)
                nc.sync.dma_start(out=hT_q[:, 0:KT // 2, :],
                                  in_=hsl[:, 0:KT // 2, :])
                nc.scalar.dma_start(out=hT_q[:, KT // 2:KT, :],
                                    in_=hsl[:, KT // 2:KT, :])

                # stream chunks for this quarter: per group g (0,1=qk / 2=v)
                # two chunks of 12 kt each
                wsts = {}
                for g in (0, 2, 1):
                    for half in range(2):
                        k0 = KC + half * (KS // 2)
                        wst = ws_pool.tile([HD, KS // 2, 512], BF, name="ws")
                        nc.sync.dma_start(
                            out=wst[:],
                            in_=wqall[:, k0:k0 + KS // 2,
                                      g * 512:(g + 1) * 512])
                        wsts[(g, half)] = wst

                def wslice(g, kt):
                    if kt < KC:
                        return wq_c[:, kt, g * 512:(g + 1) * 512]
                    half = (kt - KC) // (KS // 2)
                    return wsts[(g, half)][:, (kt - KC) % (KS // 2), :]

                # --- group fg0 (heads 0,1 q/k), then V, then fg1 ---
                for g in (0, 2, 1):
                    if g == 2:
                        # V natural: per 128-row subtile, all 4 heads
                        for ssub in range(4):
                            psv = qkv_ps.tile([HD, 512], F32, name="psv",
                                              bufs=2)
                            nc.tensor.matmul(psv[:], ones_rowb[:],
                                             sb_vbias[:],
                                             start=True, stop=False)
                            for kt in range(KT):
                                nc.tensor.matmul(
                                    psv[:],
                                    hT_q[:, kt,
                                         ssub * HD:(ssub + 1) * HD],
                                    wslice(2, kt),
                                    start=False, stop=(kt == KT - 1))
                            sk0 = s0 + ssub * HD
                            nc.scalar.activation(
                                vnat[:, :, sk0:sk0 + HD],
                                psv[:].rearrange("p (h d) -> p h d",
                                                 h=NH_LOC),
                                Act.Identity)
                    else:
                        psl = [qkv_ps.tile([HD, 512], F32, name="psqk",
                                           bufs=5) for _ in range(4)]
                        for kt in range(KT):
                            wsl = wslice(g, kt)
                            for i in range(4):
                                nc.tensor.matmul(
                                    psl[i][:],
                                    wsl[:, i * HD:(i + 1) * HD],
                                    hT_q[:, kt, :],
                                    start=(kt == 0), stop=(kt == KT - 1))
                        for i in range(4):
                            h = 2 * g + i // 2
                            dest = (qT, kTt)[i % 2][h][:, s0:s0 + 512]
                            fcol = 4 * g + i
                            nc.scalar.activation(
                                dest, psl[i][:], Act.Identity,
                                bias=sb_bqk[:, fcol:fcol + 1])

        # ---------- phase 2+3+4: attention, a2a, dense ----------
        with (
            tc.tile_pool(name="expp", bufs=4) as expp,
            tc.tile_pool(name="bcp", bufs=2) as bcp,
            tc.tile_pool(name="cxp", bufs=3) as cxp,
            tc.tile_pool(name="dns_sb", bufs=1) as dns_sb,
            tc.tile_pool(name="crecvp", bufs=2) as crecv_pool,
            tc.tile_pool(name="wd_pool", bufs=2) as wd_pool,
            tc.tile_pool(name="osb_pool", bufs=3) as osb_pool,
            tc.tile_pool(name="attn_ps", bufs=1, space="PSUM") as attn_ps,
            tc.tile_pool(name="sc_ps", bufs=3, space="PSUM") as sc_ps,
            tc.tile_pool(name="dns_ps", bufs=1, space="PSUM") as dns_ps,
        ):
            sb_bd = dns_sb.tile([1, HID], F32)
            nc.sync.dma_start(out=sb_bd[:], in_=bdense[:])

            def attention_head(h):
                for sqb in range(4):
                    q0 = sqb * 512
                    nsk = 4 * (sqb + 1)
                    ps_ctx = attn_ps.tile([HD, 512], F32, name="ps_ctx",
                                          bufs=2)
                    ps_sum = attn_ps.tile([1, 512], F32, name="ps_sum",
                                          bufs=1)
                    exs = {}

                    def flush(skt, first, last):
                        ex, c0 = exs.pop(skt)
                        nc.tensor.matmul(
                            ps_ctx[:, c0:512],
                            vnat[:, h, skt * HD:(skt + 1) * HD],
                            ex[:, c0:512], start=first, stop=last)
                        nc.tensor.matmul(
                            ps_sum[:, c0:512], ones_col[:], ex[:, c0:512],
                            start=first, stop=last)

                    for skt in range(nsk):
                        i = skt - 4 * sqb    # >= 0 on the diagonal band
                        ri = i + 15
                        c0 = i * HD if i > 0 else 0
                        ps = sc_ps.tile([HD, 512], F32, name="ps_sc")
                        nc.tensor.matmul(
                            ps[:, c0:512],
                            kTt[h][:, skt * HD:(skt + 1) * HD],
                            qT[h][:, q0 + c0:q0 + 512],
                            start=True, stop=False)
                        nc.tensor.matmul(
                            ps[:, c0:512],
                            sb_slrow[:, h * HD:(h + 1) * HD],
                            sb_negb[:, c0:512],
                            start=False, stop=True)
                        if i >= 0:
                            nc.vector.tensor_tensor(
                                ps[:, c0:c0 + HD], ps[:, c0:c0 + HD],
                                sb_mask[:], Alu.add)
                        ex = expp.tile([HD, 512], BF, name="ex")
                        nc.scalar.activation(
                            ex[:, c0:512], ps[:, c0:512], Act.Exp,
                            bias=sb_bca[:, h * NR + ri:h * NR + ri + 1])
                        exs[skt] = (ex, c0)
                        if skt >= 2:
                            flush(skt - 2, skt - 2 == 0, False)
                    for skt in (nsk - 2, nsk - 1):
                        flush(skt, skt == 0, skt == nsk - 1)

                    ps_bc = attn_ps.tile([HD, 512], F32, name="ps_bc",
                                         bufs=1)
                    sum_sb = bcp.tile([1, 512], F32, name="sum_sb")
                    nc.vector.tensor_copy(sum_sb[:], ps_sum[:])
                    nc.tensor.matmul(ps_bc[:], ones_row[:], sum_sb[:],
                                     start=True, stop=True)
                    rec_bc = bcp.tile([HD, 512], F32, name="rec_bc")
                    nc.vector.reciprocal_approx_fast(out=rec_bc[:],
                                                     in_=ps_bc[:])
                    cxc = cxp.tile([HD, 512], BF, name="cxc")
                    nc.vector.tensor_tensor(
                        cxc[:], ps_ctx[:], rec_bc[:], Alu.mult)
                    for jj in range(2):
                        j = 2 * sqb + jj
                        nc.scalar.dma_start(
                            out=a2a_in[h // 2][j, h % 2],
                            in_=cxc[:, jj * SROW:(jj + 1) * SROW])

            def dense_pass(p):
                crecv = crecv_pool.tile([HD, 16, SROW], BF, name="crecv")
                for i in range(NCORES):
                    nc.sync.dma_start(
                        out=crecv[:, 2 * i:2 * i + 2, :],
                        in_=a2a_out[p][i].rearrange("l p s -> p l s"))
                for ot in range(8):
                    o0 = ot * 512
                    wd = wd_pool.tile([HD, 16, 512], BF, name="wd")
                    nc.sync.dma_start(out=wd[:], in_=wdr[p, ot])
                    for st in range(2):
                        psd = dns_ps.tile([HD, 512], F32, name="psd")
                        if p == 0:
                            nc.tensor.matmul(
                                psd[:], ones_row[:], sb_bd[:, o0:o0 + 512],
                                start=True, stop=False)
                        for ft in range(16):
                            nc.tensor.matmul(
                                psd[:],
                                crecv[:, ft, st * HD:(st + 1) * HD],
                                wd[:, ft, :],
                                start=(p == 1 and ft == 0), stop=(ft == 15))
                        osb = osb_pool.tile([HD, 512], F32, name="osb")
                        nc.vector.tensor_copy(osb[:], psd[:])
                        if p == 0:
                            nc.sync.dma_start(
                                out=out[st * HD:(st + 1) * HD, o0:o0 + 512],
                                in_=osb[:])
                        else:
                            nc.gpsimd.dma_start(
                                out=out[st * HD:(st + 1) * HD, o0:o0 + 512],
                                in_=osb[:], accum_op=Alu.add)

            for h in (0, 1):
                attention_head(h)
            nc.gpsimd.collective_compute(
                "AllToAll", Alu.bypass,
                replica_groups=[list(range(NCORES))],
                ins=[a2a_in[0][:]], outs=[a2a_out[0][:]])
            for h in (2, 3):
                attention_head(h)
            nc.gpsimd.collective_compute(
                "AllToAll", Alu.bypass,
                replica_groups=[list(range(NCORES))],
                ins=[a2a_in[1][:]], outs=[a2a_out[1][:]])
            dense_pass(0)
            dense_pass(1)
    nc.compile()
    return nc


def _prep_shards(hidden_states, alibi, w_qkv, b_qkv, w_dense, b_dense):
    bf16 = ml_dtypes.bfloat16
    hid = np.asarray(hidden_states, dtype=np.float32).reshape(S, HID)
    hiddenT = np.ascontiguousarray(hid.T).astype(bf16)      # [HID, S]
    al = np.asarray(alibi, dtype=np.float32).reshape(NH, S)
    w = np.asarray(w_qkv, dtype=np.float32)
    b = np.asarray(b_qkv, dtype=np.float32)
    wd = np.asarray(w_dense, dtype=np.float32)
    bd = np.asarray(b_dense, dtype=np.float32)

    # fold INV_NORM into the q projections
    scale = np.ones(3 * HID, np.float32)
    for h in range(NH):
        scale[h * 3 * HD:(h * 3 * HD) + HD] = INV_NORM
    wT = np.ascontiguousarray((w * scale[:, None]).T)      # [HID, 3*HID]
    bs = b * scale

    # dense weights, transposed then per-pass/ot tiled:
    # wdr[p, ot, 128, 16, 512]; k-tile ft=2i+l <-> global head 4i+2p+l
    wdT = np.ascontiguousarray(wd.T).astype(np.float32)    # [HID(f), HID(o)]
    wdr = np.empty((2, 8, HD, 16, 512), np.float32)
    for p in range(2):
        for i in range(NCORES):
            for l in range(2):
                g = 4 * i + 2 * p + l
                blk = wdT[g * HD:(g + 1) * HD]             # [128, 4096]
                wdr[p, :, :, 2 * i + l, :] = (
                    blk.reshape(HD, 8, 512).transpose(1, 0, 2))
    wdr = wdr.astype(bf16)
    bdr = np.ascontiguousarray(bd.reshape(1, HID))

    # mask strip: 0 where a <= b, -1e9 where a > b (future key)
    a = np.arange(HD)[:, None]
    bcol = np.arange(HD)[None, :]
    maskst = np.where(a <= bcol, 0.0, -1.0e9).astype(np.float32)
    negb = np.ascontiguousarray(
        (-np.arange(512, dtype=np.float32)).reshape(1, 512)).astype(bf16)

    in_maps = []
    for c in range(NCORES):
        heads = list(range(c * NH_LOC, (c + 1) * NH_LOC))
        slopes = al[heads, 1]                              # [4]
        # wqall: [128, KT, 1536] cols = [q0 k0 q1 k1 | q2 k2 q3 k3 | v0..v3]
        cols = []
        qkcols = []
        for h in range(NH_LOC):
            base = (heads[h]) * 3 * HD
            qkcols += [list(range(base, base + HD)),
                       list(range(base + HD, base + 2 * HD))]
        for fg in range(2):
            for i in range(4):
                cols += qkcols[fg * 4 + i]
        vcols = []
        for h in range(NH_LOC):
            base = heads[h] * 3 * HD + 2 * HD
            vcols += list(range(base, base + HD))
        cols += vcols
        wq = wT[:, cols]                                   # [HID, 1536]
        wqall = np.ascontiguousarray(
            wq.reshape(KT, HD, 1536).transpose(1, 0, 2)).astype(bf16)
        bqk = np.stack(
            [bs[qkcols[f]] for f in range(8)], axis=1)     # [128, 8]
        vbias = bs[vcols].reshape(1, 512)
        biasca = np.empty((HD, NH_LOC * NR), np.float32)
        for h in range(NH_LOC):
            for ri in range(NR):
                biasca[:, h * NR + ri] = slopes[h] * (
                    (ri - 15) * HD + np.arange(HD, dtype=np.float32))
        slrow = np.repeat(slopes, HD).reshape(1, NH_LOC * HD)
        in_maps.append({
            "hiddenT": hiddenT,
            "wqall": wqall,
            "bqk": np.ascontiguousarray(bqk.astype(np.float32)),
            "vbias": np.ascontiguousarray(vbias).astype(bf16),
            "biasca": np.ascontiguousarray(biasca),
            "negbr": negb,
            "slrow": np.ascontiguousarray(slrow.astype(np.float32)).astype(
                bf16),
            "maskst": maskst,
            "wdr": wdr,
            "bdense": bdr,
        })
    return in_maps


def kernel(hidden_states, alibi, w_qkv, b_qkv, w_dense, b_dense):
    _ensure_axon_hooks()
    from concourse import bass_utils

    if "nc" not in _CACHE:
        _CACHE["nc"] = _build_nc()
    nc = _CACHE["nc"]
    in_maps = _prep_shards(hidden_states, alibi, w_qkv, b_qkv,
                           w_dense, b_dense)
    trace = bool(os.environ.get("BLOOM_TRACE"))
    res = bass_utils.run_bass_kernel_spmd(
        nc, in_maps, core_ids=list(range(NCORES)), trace=trace)
    kernel._last_results = res
    kernel._last_exec_ns = res.exec_time_ns
    outp = np.concatenate([res.results[c]["out"] for c in range(NCORES)],
                          axis=0)
    return outp.reshape(B, S, HID).astype(np.float32)


# revision 16
# speedup vs baseline: 1.1909x; 1.1606x over previous
"""BloomAttention (B=1, S=2048, HID=4096, NH=32) on 8 Trainium2 NeuronCores.

Strategy (tensor-parallel over heads):
  - Each core owns 4 heads. w_qkv/b_qkv column-sharded; INV_NORM folded into
    the q columns on host; weights shipped transposed+bf16; hidden shipped
    PRE-TRANSPOSED (hiddenT [HID, S]) in bf16 so no on-device DMA-transpose
    is needed.
  - QKV: qT/kT [d, s] come from w.T @ hT matmuls; V is produced directly in
    NATURAL [s, d] layout by swapping the matmul operands (lhsT = hT tile,
    rhs = V weight columns), so no transpose / DRAM round-trip for V.
    V bias is folded in as a K=1 ones-row matmul at accumulation start.
  - Attention in transposed-scores layout: scoresT[sk, sq] = kT.T @ qT.
    The ALiBi bias slope*(sk-sq) (with the exact per-query shift) is applied
    as: (a) a K=1 rank-1 matmul adding slope*(-sq) (per-query-constant
    rounding cancels in softmax), (b) a per-partition bias slope*(tile_off +
    sk_within_tile) fused into the exp activation on ACT (free), and (c) a
    single shared [128,128] additive -1e9 mask strip on the causal diagonal.
    Diagonal score tiles are column-narrowed (fully-masked columns skipped).
    exp on ACT; P@V and the softmax denominator are matmuls over the sk
    partitions; normalization via ones-row broadcast matmul +
    reciprocal_approx_fast.
  - AllToAll (split in two, per head-pair) swaps head-shards for
    sequence-shards; dense is split into two k-half passes, one per
    AllToAll, so pass 0 overlaps attention of heads 2,3 and the second
    collective. Pass 1 accumulates into the DRAM output via CCE accum-DMA.

Note: assumes the alibi input is the standard Bloom form alibi[h, j] =
slope_h * j (slope read from alibi[:, 1]); the reference's setup_inputs
builds exactly that.
"""

import math
import os
import sys
import types
from contextlib import ExitStack

import numpy as np
import ml_dtypes

B, S, HID, NH, HD = 1, 2048, 4096, 32, 128
NCORES = 8
NH_LOC = NH // NCORES            # 4 heads per core
SROW = S // NCORES               # 256 output rows per core
INV_NORM = 1.0 / math.sqrt(HD)
KT = HID // HD                   # 32 contraction tiles
KC = 8                           # k tiles cached in SBUF (rest streamed)
KS = KT - KC                     # streamed k tiles (24)
NR = 19                          # distinct (sk-sq)/128 tile offsets

_CACHE = {}


def _ensure_axon_hooks():
    try:
        import antenv  # noqa: F401

        extra = "/opt/trn_rl_repo/antenv"
        if os.path.isdir(extra) and extra not in antenv.__path__:
            antenv.__path__.append(extra)
        import antenv.axon_hooks  # noqa: F401
    except Exception:
        if "antenv.axon_hooks" in sys.modules:
            return
        # Functional stand-in: the axon boot code (trn_boot.py) stores the
        # NTFF profiling hook here at jax init; bass_utils reads it back.
        m = types.ModuleType("antenv.axon_hooks")
        m._hook = None

        def _set(h, _m=m):
            _m._hook = h

        m.get_axon_ntff_profile_hook = lambda _m=m: _m._hook
        m.set_axon_ntff_profile_hook = _set
        sys.modules["antenv.axon_hooks"] = m
        try:
            from trn_agent_boot.trn_boot import _ntff_profile_via_ctypes

            so = "/opt/axon/libaxon_pjrt.so"
            if os.path.isfile(so):
                hook = _ntff_profile_via_ctypes(so)
                if hook is not None:
                    m._hook = hook
        except Exception:
            pass


_ensure_axon_hooks()


def _build_nc():
    import concourse.bass as bass  # noqa: F401
    import concourse.mybir as mybir
    from concourse import bacc, bass_isa, tile

    BF = mybir.dt.bfloat16
    F32 = mybir.dt.float32
    Alu = mybir.AluOpType
    Act = mybir.ActivationFunctionType

    nc = bacc.Bacc(None, target_bir_lowering=False, num_devices=NCORES)
    with tile.TileContext(nc) as tc, ExitStack() as ctx:
        dram = ctx.enter_context(tc.tile_pool(name="dram", bufs=1, space="DRAM"))

        def din(name, shape, dt):
            return dram.tile(shape, dt, kind="ExternalInput", name=name,
                             uniquify=False)

        hiddenT = din("hiddenT", [HID, S], BF)
        # [p, kt, 1536]: cols = [fg0 qk 512 | fg1 qk 512 | v 512]
        wqall = din("wqall", [HD, KT, 1536], BF)
        bqk = din("bqk", [HD, 8], F32)          # per-feature q/k bias columns
        vbias = din("vbias", [HD, 512], F32)    # v bias bcast [4h x 128d]
        biasca = din("biasca", [HD, NH_LOC * NR], F32)  # slope*(off+a)
        negb = din("negbr", [1, 512], BF)       # -(0..511)
        slrow = din("slrow", [1, NH_LOC * HD], BF)  # slope_h replicated
        maskst = din("maskst", [HD, HD], F32)   # 0 / -1e9 strip
        wdr = din("wdr", [2, 8, HD, 16, 512], BF)
        bdense = din("bdense", [1, HID], F32)
        out = dram.tile([SROW, HID], F32, kind="ExternalOutput", name="out",
                        uniquify=False)
        a2a_in = [dram.tile([NCORES, 2, HD, SROW], BF, name=f"a2a_in{p}")
                  for p in range(2)]
        a2a_out = [dram.tile([NCORES, 2, HD, SROW], BF, name=f"a2a_out{p}")
                   for p in range(2)]

        # ---------- persistent SBUF ----------
        const = ctx.enter_context(tc.tile_pool(name="const", bufs=1))
        sb_bqk = const.tile([HD, 8], F32)
        nc.sync.dma_start(out=sb_bqk[:], in_=bqk[:])
        sb_vbias = const.tile([HD, 512], F32)
        nc.sync.dma_start(out=sb_vbias[:], in_=vbias[:])
        sb_bca = const.tile([HD, NH_LOC * NR], F32)
        nc.sync.dma_start(out=sb_bca[:], in_=biasca[:])
        sb_negb = const.tile([1, 512], BF)
        nc.sync.dma_start(out=sb_negb[:], in_=negb[:])
        sb_slrow = const.tile([1, NH_LOC * HD], BF)
        nc.sync.dma_start(out=sb_slrow[:], in_=slrow[:])
        sb_mask = const.tile([HD, HD], F32)
        nc.sync.dma_start(out=sb_mask[:], in_=maskst[:])
        ones_row = const.tile([1, HD], F32)
        nc.vector.memset(ones_row[:], 1.0)

        persist = ctx.enter_context(tc.tile_pool(name="persist", bufs=1))
        qT = [persist.tile([HD, S], BF, name=f"qT{h}") for h in range(NH_LOC)]
        kTt = [persist.tile([HD, S], BF, name=f"kT{h}") for h in range(NH_LOC)]
        vnat = persist.tile([HD, NH_LOC, S], BF, name="vnat")

        # ---------- phase 1: QKV ----------
        with (
            tc.tile_pool(name="wqc", bufs=1) as wqc_pool,
            tc.tile_pool(name="wstream", bufs=4) as ws_pool,
            tc.tile_pool(name="hT", bufs=2) as hT_pool,
            tc.tile_pool(name="qkv_ps", bufs=1, space="PSUM") as qkv_ps,
        ):
            wq_c = wqc_pool.tile([HD, KC, 1536], BF)
            nc.sync.dma_start(out=wq_c[:], in_=wqall[:, :KC, :])

            for sq in range(4):  # s-quarters of 512
                s0 = sq * 512
                hT_q = hT_pool.tile([HD, KT, 512], BF, name="hT_q")
                hsl = hiddenT[:, s0:s0 + 512].rearrange(
                    "(k p) s -> p k s", p=HD)
                nc.sync.dma_start(out=hT_q[:, 0:KT // 2, :],
                                  in_=hsl[:, 0:KT // 2, :])
                nc.scalar.dma_start(out=hT_q[:, KT // 2:KT, :],
                                    in_=hsl[:, KT // 2:KT, :])

                # stream chunks for this quarter: per group g (0,1=qk / 2=v)
                # two chunks of 12 kt each
                wsts = {}
                for g in (0, 2, 1):
                    for half in range(2):
                        k0 = KC + half * (KS // 2)
                        wst = ws_pool.tile([HD, KS // 2, 512], BF, name="ws")
                        nc.sync.dma_start(
                            out=wst[:],
                            in_=wqall[:, k0:k0 + KS // 2,
                                      g * 512:(g + 1) * 512])
                        wsts[(g, half)] = wst

                def wslice(g, kt):
                    if kt < KC:
                        return wq_c[:, kt, g * 512:(g + 1) * 512]
                    half = (kt - KC) // (KS // 2)
                    return wsts[(g, half)][:, (kt - KC) % (KS // 2), :]

                # --- group fg0 (heads 0,1 q/k), then V, then fg1 ---
                for g in (0, 2, 1):
                    if g == 2:
                        # V natural: per 128-row subtile, all 4 heads
                        for ssub in range(4):
                            psv = qkv_ps.tile([HD, 512], F32, name="psv",
                                              bufs=2)
                            for kt in range(KT):
                                nc.tensor.matmul(
                                    psv[:],
                                    hT_q[:, kt,
                                         ssub * HD:(ssub + 1) * HD],
                                    wslice(2, kt),
                                    start=(kt == 0), stop=(kt == KT - 1))
                            sk0 = s0 + ssub * HD
                            nc.vector.tensor_tensor(
                                vnat[:, :, sk0:sk0 + HD],
                                psv[:].rearrange("p (h d) -> p h d",
                                                 h=NH_LOC),
                                sb_vbias[:].rearrange("p (h d) -> p h d",
                                                      h=NH_LOC),
                                Alu.add)
                    else:
                        psl = [qkv_ps.tile([HD, 512], F32, name="psqk",
                                           bufs=5) for _ in range(4)]
                        for kt in range(KT):
                            wsl = wslice(g, kt)
                            for i in range(4):
                                nc.tensor.matmul(
                                    psl[i][:],
                                    wsl[:, i * HD:(i + 1) * HD],
                                    hT_q[:, kt, :],
                                    start=(kt == 0), stop=(kt == KT - 1))
                        for i in range(4):
                            h = 2 * g + i // 2
                            dest = (qT, kTt)[i % 2][h][:, s0:s0 + 512]
                            fcol = 4 * g + i
                            nc.scalar.activation(
                                dest, psl[i][:], Act.Identity,
                                bias=sb_bqk[:, fcol:fcol + 1])

        # ---------- phase 2+3+4: attention, a2a, dense ----------
        with (
            tc.tile_pool(name="expp", bufs=2) as expp,
            tc.tile_pool(name="sump", bufs=2) as sump,
            tc.tile_pool(name="bcp", bufs=2) as bcp,
            tc.tile_pool(name="cxp", bufs=3) as cxp,
            tc.tile_pool(name="dns_sb", bufs=1) as dns_sb,
            tc.tile_pool(name="crecvp", bufs=2) as crecv_pool,
            tc.tile_pool(name="wd_pool", bufs=2) as wd_pool,
            tc.tile_pool(name="osb_pool", bufs=3) as osb_pool,
            tc.tile_pool(name="attn_ps", bufs=1, space="PSUM") as attn_ps,
            tc.tile_pool(name="sc_ps", bufs=4, space="PSUM") as sc_ps,
            tc.tile_pool(name="dns_ps", bufs=2, space="PSUM") as dns_ps,
        ):
            sb_bd = dns_sb.tile([1, HID], F32)
            nc.sync.dma_start(out=sb_bd[:], in_=bdense[:])

            def attention_head(h):
                for sqb in range(4):
                    q0 = sqb * 512
                    nsk = 4 * (sqb + 1)
                    ps_ctx = attn_ps.tile([HD, 512], F32, name="ps_ctx",
                                          bufs=2)
                    # exp tiles for this block + running DVE accumulator of
                    # their per-partition sums (partition-reduced at the end)
                    exb = expp.tile([HD, 16, 512], BF, name="exb")
                    acc = sump.tile([HD, 512], F32, name="acc")
                    c0s = {}

                    def flush(skt, first, last):
                        c0 = c0s[skt]
                        nc.tensor.matmul(
                            ps_ctx[:, c0:512],
                            vnat[:, h, skt * HD:(skt + 1) * HD],
                            exb[:, skt, c0:512], start=first, stop=last)

                    for skt in range(nsk):
                        i = skt - 4 * sqb    # >= 0 on the diagonal band
                        ri = i + 15
                        c0 = i * HD if i > 0 else 0
                        ps = sc_ps.tile([HD, 512], F32, name="ps_sc")
                        nc.tensor.matmul(
                            ps[:, c0:512],
                            kTt[h][:, skt * HD:(skt + 1) * HD],
                            qT[h][:, q0 + c0:q0 + 512],
                            start=True, stop=False)
                        nc.tensor.matmul(
                            ps[:, c0:512],
                            sb_slrow[:, h * HD:(h + 1) * HD],
                            sb_negb[:, c0:512],
                            start=False, stop=True)
                        if i >= 0:
                            nc.vector.tensor_tensor(
                                ps[:, c0:c0 + HD], ps[:, c0:c0 + HD],
                                sb_mask[:], Alu.add)
                        nc.scalar.activation(
                            exb[:, skt, c0:512], ps[:, c0:512], Act.Exp,
                            bias=sb_bca[:, h * NR + ri:h * NR + ri + 1])
                        if skt == 0:
                            nc.vector.tensor_copy(acc[:], exb[:, 0, :])
                        else:
                            nc.vector.tensor_tensor(
                                acc[:, c0:512], acc[:, c0:512],
                                exb[:, skt, c0:512], Alu.add)
                        c0s[skt] = c0
                        if skt >= 2:
                            flush(skt - 2, skt - 2 == 0, False)
                    for skt in (nsk - 2, nsk - 1):
                        flush(skt, skt == 0, skt == nsk - 1)

                    den_bc = bcp.tile([HD, 512], F32, name="den_bc")
                    nc.gpsimd.partition_all_reduce(
                        den_bc[:], acc[:], HD, bass_isa.ReduceOp.add)
                    rec_bc = bcp.tile([HD, 512], F32, name="rec_bc")
                    nc.vector.reciprocal_approx_fast(out=rec_bc[:],
                                                     in_=den_bc[:])
                    cxc = cxp.tile([HD, 512], BF, name="cxc")
                    nc.vector.tensor_tensor(
                        cxc[:], ps_ctx[:], rec_bc[:], Alu.mult)
                    for jj in range(2):
                        j = 2 * sqb + jj
                        nc.scalar.dma_start(
                            out=a2a_in[h // 2][j, h % 2],
                            in_=cxc[:, jj * SROW:(jj + 1) * SROW])

            def dense_pass(p):
                crecv = crecv_pool.tile([HD, 16, SROW], BF, name="crecv")
                for i in range(NCORES):
                    nc.sync.dma_start(
                        out=crecv[:, 2 * i:2 * i + 2, :],
                        in_=a2a_out[p][i].rearrange("l p s -> p l s"))
                for ot in range(8):
                    o0 = ot * 512
                    wd = wd_pool.tile([HD, 16, 512], BF, name="wd")
                    nc.sync.dma_start(out=wd[:], in_=wdr[p, ot])
                    for st in range(2):
                        psd = dns_ps.tile([HD, 512], F32, name="psd")
                        if p == 0:
                            nc.tensor.matmul(
                                psd[:], ones_row[:], sb_bd[:, o0:o0 + 512],
                                start=True, stop=False)
                        for ft in range(16):
                            nc.tensor.matmul(
                                psd[:],
                                crecv[:, ft, st * HD:(st + 1) * HD],
                                wd[:, ft, :],
                                start=(p == 1 and ft == 0), stop=(ft == 15))
                        osb = osb_pool.tile([HD, 512], F32, name="osb")
                        nc.vector.tensor_copy(osb[:], psd[:])
                        if p == 0:
                            nc.sync.dma_start(
                                out=out[st * HD:(st + 1) * HD, o0:o0 + 512],
                                in_=osb[:])
                        else:
                            nc.gpsimd.dma_start(
                                out=out[st * HD:(st + 1) * HD, o0:o0 + 512],
                                in_=osb[:], accum_op=Alu.add)

            for h in (0, 1):
                attention_head(h)
            nc.gpsimd.collective_compute(
                "AllToAll", Alu.bypass,
                replica_groups=[list(range(NCORES))],
                ins=[a2a_in[0][:]], outs=[a2a_out[0][:]])
            for h in (2, 3):
                attention_head(h)
            nc.gpsimd.collective_compute(
                "AllToAll", Alu.bypass,
                replica_groups=[list(range(NCORES))],
                ins=[a2a_in[1][:]], outs=[a2a_out[1][:]])
            dense_pass(0)
            dense_pass(1)
    nc.compile()
    return nc


def _prep_shards(hidden_states, alibi, w_qkv, b_qkv, w_dense, b_dense):
    bf16 = ml_dtypes.bfloat16
    hid = np.asarray(hidden_states, dtype=np.float32).reshape(S, HID)
    hiddenT = np.ascontiguousarray(hid.T).astype(bf16)      # [HID, S]
    al = np.asarray(alibi, dtype=np.float32).reshape(NH, S)
    w = np.asarray(w_qkv, dtype=np.float32)
    b = np.asarray(b_qkv, dtype=np.float32)
    wd = np.asarray(w_dense, dtype=np.float32)
    bd = np.asarray(b_dense, dtype=np.float32)

    # fold INV_NORM into the q projections
    scale = np.ones(3 * HID, np.float32)
    for h in range(NH):
        scale[h * 3 * HD:(h * 3 * HD) + HD] = INV_NORM
    wT = np.ascontiguousarray((w * scale[:, None]).T)      # [HID, 3*HID]
    bs = b * scale

    # dense weights, transposed then per-pass/ot tiled:
    # wdr[p, ot, 128, 16, 512]; k-tile ft=2i+l <-> global head 4i+2p+l
    wdT = np.ascontiguousarray(wd.T).astype(np.float32)    # [HID(f), HID(o)]
    wdr = np.empty((2, 8, HD, 16, 512), np.float32)
    for p in range(2):
        for i in range(NCORES):
            for l in range(2):
                g = 4 * i + 2 * p + l
                blk = wdT[g * HD:(g + 1) * HD]             # [128, 4096]
                wdr[p, :, :, 2 * i + l, :] = (
                    blk.reshape(HD, 8, 512).transpose(1, 0, 2))
    wdr = wdr.astype(bf16)
    bdr = np.ascontiguousarray(bd.reshape(1, HID))

    # mask strip: 0 where a <= b, -1e9 where a > b (future key)
    a = np.arange(HD)[:, None]
    bcol = np.arange(HD)[None, :]
    maskst = np.where(a <= bcol, 0.0, -1.0e9).astype(np.float32)
    negb = np.ascontiguousarray(
        (-np.arange(512, dtype=np.float32)).reshape(1, 512)).astype(bf16)

    in_maps = []
    for c in range(NCORES):
        heads = list(range(c * NH_LOC, (c + 1) * NH_LOC))
        slopes = al[heads, 1]                              # [4]
        # wqall: [128, KT, 1536] cols = [q0 k0 q1 k1 | q2 k2 q3 k3 | v0..v3]
        cols = []
        qkcols = []
        for h in range(NH_LOC):
            base = (heads[h]) * 3 * HD
            qkcols += [list(range(base, base + HD)),
                       list(range(base + HD, base + 2 * HD))]
        for fg in range(2):
            for i in range(4):
                cols += qkcols[fg * 4 + i]
        vcols = []
        for h in range(NH_LOC):
            base = heads[h] * 3 * HD + 2 * HD
            vcols += list(range(base, base + HD))
        cols += vcols
        wq = wT[:, cols]                                   # [HID, 1536]
        wqall = np.ascontiguousarray(
            wq.reshape(KT, HD, 1536).transpose(1, 0, 2)).astype(bf16)
        bqk = np.stack(
            [bs[qkcols[f]] for f in range(8)], axis=1)     # [128, 8]
        vbias = np.broadcast_to(bs[vcols].reshape(1, 512), (HD, 512))
        biasca = np.empty((HD, NH_LOC * NR), np.float32)
        for h in range(NH_LOC):
            for ri in range(NR):
                biasca[:, h * NR + ri] = slopes[h] * (
                    (ri - 15) * HD + np.arange(HD, dtype=np.float32))
        slrow = np.repeat(slopes, HD).reshape(1, NH_LOC * HD)
        in_maps.append({
            "hiddenT": hiddenT,
            "wqall": wqall,
            "bqk": np.ascontiguousarray(bqk.astype(np.float32)),
            "vbias": np.ascontiguousarray(vbias.astype(np.float32)),
            "biasca": np.ascontiguousarray(biasca),
            "negbr": negb,
            "slrow": np.ascontiguousarray(slrow.astype(np.float32)).astype(
                bf16),
            "maskst": maskst,
            "wdr": wdr,
            "bdense": bdr,
        })
    return in_maps


def kernel(hidden_states, alibi, w_qkv, b_qkv, w_dense, b_dense):
    _ensure_axon_hooks()
    from concourse import bass_utils

    if "nc" not in _CACHE:
        _CACHE["nc"] = _build_nc()
    nc = _CACHE["nc"]
    in_maps = _prep_shards(hidden_states, alibi, w_qkv, b_qkv,
                           w_dense, b_dense)
    trace = bool(os.environ.get("BLOOM_TRACE"))
    res = bass_utils.run_bass_kernel_spmd(
        nc, in_maps, core_ids=list(range(NCORES)), trace=trace)
    kernel._last_results = res
    kernel._last_exec_ns = res.exec_time_ns
    outp = np.concatenate([res.results[c]["out"] for c in range(NCORES)],
                          axis=0)
    return outp.reshape(B, S, HID).astype(np.float32)


# revision 25
# speedup vs baseline: 1.2698x; 1.0663x over previous
"""BloomAttention (B=1, S=2048, HID=4096, NH=32) on 8 Trainium2 NeuronCores.

Strategy (tensor-parallel over heads):
  - Each core owns 4 heads. w_qkv/b_qkv column-sharded; INV_NORM folded into
    the q columns on host; weights shipped transposed+bf16; hidden shipped
    PRE-TRANSPOSED (hiddenT [HID, S]) in bf16 so no on-device DMA-transpose
    is needed.
  - QKV: qT/kT [d, s] come from w.T @ hT matmuls; V is produced directly in
    NATURAL [s, d] layout by swapping the matmul operands (lhsT = hT tile,
    rhs = V weight columns), so no transpose / DRAM round-trip for V.
    V bias is folded in as a K=1 ones-row matmul at accumulation start.
  - Attention in transposed-scores layout: scoresT[sk, sq] = kT.T @ qT.
    The ALiBi bias slope*(sk-sq) (with the exact per-query shift) is applied
    as: (a) a K=1 rank-1 matmul adding slope*(-sq) (per-query-constant
    rounding cancels in softmax), (b) a per-partition bias slope*(tile_off +
    sk_within_tile) fused into the exp activation on ACT (free), and (c) a
    single shared [128,128] additive -1e9 mask strip on the causal diagonal.
    Diagonal score tiles are column-narrowed (fully-masked columns skipped).
    exp on ACT; P@V and the softmax denominator are matmuls over the sk
    partitions; normalization via ones-row broadcast matmul +
    reciprocal_approx_fast.
  - AllToAll (split in two, per head-pair) swaps head-shards for
    sequence-shards; dense is split into two k-half passes, one per
    AllToAll, so pass 0 overlaps attention of heads 2,3 and the second
    collective. Pass 1 accumulates into the DRAM output via CCE accum-DMA.

Note: assumes the alibi input is the standard Bloom form alibi[h, j] =
slope_h * j (slope read from alibi[:, 1]); the reference's setup_inputs
builds exactly that.
"""

import math
import os
import sys
import types
from contextlib import ExitStack

import numpy as np
import ml_dtypes

B, S, HID, NH, HD = 1, 2048, 4096, 32, 128
NCORES = 8
NH_LOC = NH // NCORES            # 4 heads per core
SROW = S // NCORES               # 256 output rows per core
INV_NORM = 1.0 / math.sqrt(HD)
KT = HID // HD                   # 32 contraction tiles
KC = 8                           # k tiles cached in SBUF (rest streamed)
KS = KT - KC                     # streamed k tiles (24)
NR = 19                          # distinct (sk-sq)/128 tile offsets

_CACHE = {}


def _ensure_axon_hooks():
    try:
        import antenv  # noqa: F401

        extra = "/opt/trn_rl_repo/antenv"
        if os.path.isdir(extra) and extra not in antenv.__path__:
            antenv.__path__.append(extra)
        import antenv.axon_hooks  # noqa: F401
    except Exception:
        if "antenv.axon_hooks" in sys.modules:
            return
        # Functional stand-in: the axon boot code (trn_boot.py) stores the
        # NTFF profiling hook here at jax init; bass_utils reads it back.
        m = types.ModuleType("antenv.axon_hooks")
        m._hook = None

        def _set(h, _m=m):
            _m._hook = h

        m.get_axon_ntff_profile_hook = lambda _m=m: _m._hook
        m.set_axon_ntff_profile_hook = _set
        sys.modules["antenv.axon_hooks"] = m
        try:
            from trn_agent_boot.trn_boot import _ntff_profile_via_ctypes

            so = "/opt/axon/libaxon_pjrt.so"
            if os.path.isfile(so):
                hook = _ntff_profile_via_ctypes(so)
                if hook is not None:
                    m._hook = hook
        except Exception:
            pass


_ensure_axon_hooks()


def _build_nc():
    import concourse.bass as bass  # noqa: F401
    import concourse.mybir as mybir
    from concourse import bacc, bass_isa, tile

    BF = mybir.dt.bfloat16
    F32 = mybir.dt.float32
    Alu = mybir.AluOpType
    Act = mybir.ActivationFunctionType

    nc = bacc.Bacc(None, target_bir_lowering=False, num_devices=NCORES)
    with tile.TileContext(nc) as tc, ExitStack() as ctx:
        dram = ctx.enter_context(tc.tile_pool(name="dram", bufs=1, space="DRAM"))

        def din(name, shape, dt):
            return dram.tile(shape, dt, kind="ExternalInput", name=name,
                             uniquify=False)

        hiddenT = din("hiddenT", [HID, S], BF)
        # [p, kt, 1536]: cols = [fg0 qk 512 | fg1 qk 512 | v 512]
        wqall = din("wqall", [HD, KT, 1536], BF)
        bqk = din("bqk", [HD, 8], F32)          # per-feature q/k bias columns
        vbias = din("vbias", [HD, 512], F32)    # v bias bcast [4h x 128d]
        biasca = din("biasca", [HD, NH_LOC * NR], F32)  # slope*(off+a)
        negbbc = din("negbbc", [HD, 512], BF)   # -(0..511) bcast rows
        slmat = din("slmat", [HD, NH_LOC * HD], BF)  # slope_h/128 blocks
        maskst = din("maskst", [HD, HD], F32)   # 0 / -1e9 strip
        wdr = din("wdr", [NH_LOC, 8, HD, 8, 512], BF)
        bdense = din("bdense", [1, HID], F32)
        out = dram.tile([SROW, HID], F32, kind="ExternalOutput", name="out",
                        uniquify=False)
        a2a_in = [dram.tile([NCORES, HD, SROW], BF, name=f"a2a_in{p}")
                  for p in range(NH_LOC)]
        a2a_out = [dram.tile([NCORES, HD, SROW], BF, name=f"a2a_out{p}")
                   for p in range(NH_LOC)]

        # ---------- persistent SBUF ----------
        const = ctx.enter_context(tc.tile_pool(name="const", bufs=1))
        sb_bqk = const.tile([HD, 8], F32)
        nc.sync.dma_start(out=sb_bqk[:], in_=bqk[:])
        sb_vbias = const.tile([HD, 512], F32)
        nc.sync.dma_start(out=sb_vbias[:], in_=vbias[:])
        sb_bca = const.tile([HD, NH_LOC * NR], F32)
        nc.sync.dma_start(out=sb_bca[:], in_=biasca[:])
        sb_negb = const.tile([HD, 512], BF)
        nc.sync.dma_start(out=sb_negb[:], in_=negbbc[:])
        sb_slmat = const.tile([HD, NH_LOC * HD], BF)
        nc.sync.dma_start(out=sb_slmat[:], in_=slmat[:])
        sb_mask = const.tile([HD, HD], F32)
        nc.sync.dma_start(out=sb_mask[:], in_=maskst[:])
        ones_row = const.tile([1, HD], F32)
        nc.vector.memset(ones_row[:], 1.0)

        persist = ctx.enter_context(tc.tile_pool(name="persist", bufs=1))
        qT = [persist.tile([HD, S], BF, name=f"qT{h}") for h in range(NH_LOC)]
        kTt = [persist.tile([HD, S], BF, name=f"kT{h}") for h in range(NH_LOC)]
        vnat = persist.tile([HD, NH_LOC, S], BF, name="vnat")

        # ---------- phase 1: QKV ----------
        with (
            tc.tile_pool(name="wqc", bufs=1) as wqc_pool,
            tc.tile_pool(name="wstream", bufs=4) as ws_pool,
            tc.tile_pool(name="hT", bufs=2) as hT_pool,
            tc.tile_pool(name="qkv_ps", bufs=1, space="PSUM") as qkv_ps,
        ):
            wq_c = wqc_pool.tile([HD, KC, 1536], BF)

            for sq in range(4):  # s-quarters of 512
                s0 = sq * 512
                hT_q = hT_pool.tile([HD, KT, 512], BF, name="hT_q")
                hsl = hiddenT[:, s0:s0 + 512].rearrange(
                    "(k p) s -> p k s", p=HD)
                nc.sync.dma_start(out=hT_q[:, 0:KT // 2, :],
                                  in_=hsl[:, 0:KT // 2, :])
                nc.scalar.dma_start(out=hT_q[:, KT // 2:KT, :],
                                    in_=hsl[:, KT // 2:KT, :])
                if sq == 0:
                    # cached weights: issued after the first hT DMAs, split
                    # so early k-tiles land first
                    nc.sync.dma_start(out=wq_c[:, 0:KC // 2, :],
                                      in_=wqall[:, 0:KC // 2, :])
                    nc.scalar.dma_start(out=wq_c[:, KC // 2:KC, :],
                                        in_=wqall[:, KC // 2:KC, :])

                # stream chunks for this quarter: per group g (0,1=qk / 2=v)
                # two chunks of 12 kt each
                wsts = {}
                for g in (0, 2, 1):
                    for half in range(2):
                        k0 = KC + half * (KS // 2)
                        wst = ws_pool.tile([HD, KS // 2, 512], BF, name="ws")
                        nc.sync.dma_start(
                            out=wst[:],
                            in_=wqall[:, k0:k0 + KS // 2,
                                      g * 512:(g + 1) * 512])
                        wsts[(g, half)] = wst

                def wslice(g, kt):
                    if kt < KC:
                        return wq_c[:, kt, g * 512:(g + 1) * 512]
                    half = (kt - KC) // (KS // 2)
                    return wsts[(g, half)][:, (kt - KC) % (KS // 2), :]

                # --- group fg0 (heads 0,1 q/k), then V, then fg1 ---
                for g in (0, 2, 1):
                    if g == 2:
                        # V natural: per 128-row subtile, all 4 heads
                        for ssub in range(4):
                            psv = qkv_ps.tile([HD, 512], F32, name="psv",
                                              bufs=2)
                            for kt in range(KT):
                                nc.tensor.matmul(
                                    psv[:],
                                    hT_q[:, kt,
                                         ssub * HD:(ssub + 1) * HD],
                                    wslice(2, kt),
                                    start=(kt == 0), stop=(kt == KT - 1))
                            sk0 = s0 + ssub * HD
                            nc.vector.tensor_tensor(
                                vnat[:, :, sk0:sk0 + HD],
                                psv[:].rearrange("p (h d) -> p h d",
                                                 h=NH_LOC),
                                sb_vbias[:].rearrange("p (h d) -> p h d",
                                                      h=NH_LOC),
                                Alu.add)
                    else:
                        psl = [qkv_ps.tile([HD, 512], F32, name="psqk",
                                           bufs=5) for _ in range(4)]
                        for kt in range(KT):
                            wsl = wslice(g, kt)
                            for i in range(4):
                                nc.tensor.matmul(
                                    psl[i][:],
                                    wsl[:, i * HD:(i + 1) * HD],
                                    hT_q[:, kt, :],
                                    start=(kt == 0), stop=(kt == KT - 1))
                        for i in range(4):
                            h = 2 * g + i // 2
                            dest = (qT, kTt)[i % 2][h][:, s0:s0 + 512]
                            fcol = 4 * g + i
                            nc.scalar.activation(
                                dest, psl[i][:], Act.Identity,
                                bias=sb_bqk[:, fcol:fcol + 1])

        # ---------- phase 2+3+4: attention, a2a, dense ----------
        with (
            tc.tile_pool(name="expp", bufs=2) as expp,
            tc.tile_pool(name="sump", bufs=2) as sump,
            tc.tile_pool(name="bcp", bufs=2) as bcp,
            tc.tile_pool(name="cxp", bufs=3) as cxp,
            tc.tile_pool(name="dns_sb", bufs=1) as dns_sb,
            tc.tile_pool(name="crecvp", bufs=2) as crecv_pool,
            tc.tile_pool(name="wd_pool", bufs=2) as wd_pool,
            tc.tile_pool(name="osb_pool", bufs=1) as osb_pool,
            tc.tile_pool(name="attn_ps", bufs=1, space="PSUM") as attn_ps,
            tc.tile_pool(name="sc_ps", bufs=4, space="PSUM") as sc_ps,
            tc.tile_pool(name="dns_ps", bufs=2, space="PSUM") as dns_ps,
        ):
            sb_bd = dns_sb.tile([1, HID], F32)
            nc.sync.dma_start(out=sb_bd[:], in_=bdense[:])

            def attention_head(h):
                for sqb in range(4):
                    q0 = sqb * 512
                    nsk = 4 * (sqb + 1)
                    ps_ctx = attn_ps.tile([HD, 512], F32, name="ps_ctx",
                                          bufs=2)
                    # exp tiles for this block + running DVE accumulator of
                    # their per-partition sums (partition-reduced at the end)
                    exb = expp.tile([HD, 16, 512], BF, name="exb")
                    acc = sump.tile([HD, 512], F32, name="acc")
                    c0s = {}

                    def flush(skt, first, last):
                        c0 = c0s[skt]
                        nc.tensor.matmul(
                            ps_ctx[:, c0:512],
                            vnat[:, h, skt * HD:(skt + 1) * HD],
                            exb[:, skt, c0:512], start=first, stop=last)

                    for skt in range(nsk):
                        i = skt - 4 * sqb    # >= 0 on the diagonal band
                        ri = i + 15
                        c0 = i * HD if i > 0 else 0
                        ps = sc_ps.tile([HD, 512], F32, name="ps_sc")
                        nc.tensor.matmul(
                            ps[:, c0:512],
                            kTt[h][:, skt * HD:(skt + 1) * HD],
                            qT[h][:, q0 + c0:q0 + 512],
                            start=True, stop=False)
                        nc.tensor.matmul(
                            ps[:, c0:512],
                            sb_slmat[:, h * HD:(h + 1) * HD],
                            sb_negb[:, c0:512],
                            start=False, stop=True)
                        if i >= 0:
                            nc.vector.tensor_tensor(
                                ps[:, c0:c0 + HD], ps[:, c0:c0 + HD],
                                sb_mask[:], Alu.add)
                        nc.scalar.activation(
                            exb[:, skt, c0:512], ps[:, c0:512], Act.Exp,
                            bias=sb_bca[:, h * NR + ri:h * NR + ri + 1])
                        if skt == 0:
                            nc.vector.tensor_copy(acc[:], exb[:, 0, :])
                        else:
                            nc.vector.tensor_tensor(
                                acc[:, c0:512], acc[:, c0:512],
                                exb[:, skt, c0:512], Alu.add)
                        c0s[skt] = c0
                        if skt >= 2:
                            flush(skt - 2, skt - 2 == 0, False)
                    for skt in (nsk - 2, nsk - 1):
                        flush(skt, skt == 0, skt == nsk - 1)

                    den_bc = bcp.tile([HD, 512], F32, name="den_bc")
                    nc.gpsimd.partition_all_reduce(
                        den_bc[:], acc[:], HD, bass_isa.ReduceOp.add)
                    rec_bc = bcp.tile([HD, 512], F32, name="rec_bc")
                    nc.vector.reciprocal_approx_fast(out=rec_bc[:],
                                                     in_=den_bc[:])
                    cxc = cxp.tile([HD, 512], BF, name="cxc")
                    nc.vector.tensor_tensor(
                        cxc[:], ps_ctx[:], rec_bc[:], Alu.mult)
                    for jj in range(2):
                        j = 2 * sqb + jj
                        nc.scalar.dma_start(
                            out=a2a_in[h][j],
                            in_=cxc[:, jj * SROW:(jj + 1) * SROW])

            osbs = {}

            def dense_pass(p):
                crecv = crecv_pool.tile([HD, NCORES, SROW], BF,
                                        name="crecv")
                for i in range(NCORES):
                    nc.sync.dma_start(out=crecv[:, i, :],
                                      in_=a2a_out[p][i])
                for ot in range(8):
                    o0 = ot * 512
                    wd = wd_pool.tile([HD, 8, 512], BF, name="wd")
                    nc.sync.dma_start(out=wd[:], in_=wdr[p, ot])
                    for st in range(2):
                        psd = dns_ps.tile([HD, 512], F32, name="psd")
                        if p == 0:
                            nc.tensor.matmul(
                                psd[:], ones_row[:], sb_bd[:, o0:o0 + 512],
                                start=True, stop=False)
                        for ft in range(8):
                            nc.tensor.matmul(
                                psd[:],
                                crecv[:, ft, st * HD:(st + 1) * HD],
                                wd[:, ft, :],
                                start=(p > 0 and ft == 0), stop=(ft == 7))
                        if p == 0:
                            osb = osb_pool.tile([HD, 512], F32,
                                                name=f"osb{ot}_{st}")
                            osbs[(ot, st)] = osb
                            nc.vector.tensor_copy(osb[:], psd[:])
                        else:
                            osb = osbs[(ot, st)]
                            nc.vector.tensor_tensor(
                                osb[:], osb[:], psd[:], Alu.add)
                        if p == NH_LOC - 1:
                            nc.sync.dma_start(
                                out=out[st * HD:(st + 1) * HD, o0:o0 + 512],
                                in_=osb[:])

            for h in range(NH_LOC):
                attention_head(h)
                nc.gpsimd.collective_compute(
                    "AllToAll", Alu.bypass,
                    replica_groups=[list(range(NCORES))],
                    ins=[a2a_in[h][:]], outs=[a2a_out[h][:]])
            for p in range(NH_LOC):
                dense_pass(p)
    nc.compile()
    return nc


def _prep_shards(hidden_states, alibi, w_qkv, b_qkv, w_dense, b_dense):
    bf16 = ml_dtypes.bfloat16
    hid = np.asarray(hidden_states, dtype=np.float32).reshape(S, HID)
    hiddenT = np.ascontiguousarray(hid.T).astype(bf16)      # [HID, S]
    al = np.asarray(alibi, dtype=np.float32).reshape(NH, S)
    w = np.asarray(w_qkv, dtype=np.float32)
    b = np.asarray(b_qkv, dtype=np.float32)
    wd = np.asarray(w_dense, dtype=np.float32)
    bd = np.asarray(b_dense, dtype=np.float32)

    # fold INV_NORM into the q projections
    scale = np.ones(3 * HID, np.float32)
    for h in range(NH):
        scale[h * 3 * HD:(h * 3 * HD) + HD] = INV_NORM
    wT = np.ascontiguousarray((w * scale[:, None]).T)      # [HID, 3*HID]
    bs = b * scale

    # dense weights, transposed then per-pass/ot tiled:
    # wdr[p, ot, 128, 8, 512]; k-tile ft=i <-> global head 4i+p
    wdT = np.ascontiguousarray(wd.T).astype(np.float32)    # [HID(f), HID(o)]
    wdr = np.empty((NH_LOC, 8, HD, NCORES, 512), np.float32)
    for p in range(NH_LOC):
        for i in range(NCORES):
            g = 4 * i + p
            blk = wdT[g * HD:(g + 1) * HD]                 # [128, 4096]
            wdr[p, :, :, i, :] = blk.reshape(HD, 8, 512).transpose(1, 0, 2)
    wdr = wdr.astype(bf16)
    bdr = np.ascontiguousarray(bd.reshape(1, HID))

    # mask strip: 0 where a <= b, -1e9 where a > b (future key)
    a = np.arange(HD)[:, None]
    bcol = np.arange(HD)[None, :]
    maskst = np.where(a <= bcol, 0.0, -1.0e9).astype(np.float32)
    negbbc = np.ascontiguousarray(np.broadcast_to(
        -np.arange(512, dtype=np.float32).reshape(1, 512),
        (HD, 512))).astype(bf16)

    in_maps = []
    for c in range(NCORES):
        heads = list(range(c * NH_LOC, (c + 1) * NH_LOC))
        slopes = al[heads, 1]                              # [4]
        # wqall: [128, KT, 1536] cols = [q0 k0 q1 k1 | q2 k2 q3 k3 | v0..v3]
        cols = []
        qkcols = []
        for h in range(NH_LOC):
            base = (heads[h]) * 3 * HD
            qkcols += [list(range(base, base + HD)),
                       list(range(base + HD, base + 2 * HD))]
        for fg in range(2):
            for i in range(4):
                cols += qkcols[fg * 4 + i]
        vcols = []
        for h in range(NH_LOC):
            base = heads[h] * 3 * HD + 2 * HD
            vcols += list(range(base, base + HD))
        cols += vcols
        wq = wT[:, cols]                                   # [HID, 1536]
        wqall = np.ascontiguousarray(
            wq.reshape(KT, HD, 1536).transpose(1, 0, 2)).astype(bf16)
        bqk = np.stack(
            [bs[qkcols[f]] for f in range(8)], axis=1)     # [128, 8]
        vbias = np.broadcast_to(bs[vcols].reshape(1, 512), (HD, 512))
        biasca = np.empty((HD, NH_LOC * NR), np.float32)
        for h in range(NH_LOC):
            for ri in range(NR):
                biasca[:, h * NR + ri] = slopes[h] * (
                    (ri - 15) * HD + np.arange(HD, dtype=np.float32))
        slmat = np.repeat(slopes / HD, HD).reshape(1, NH_LOC * HD)
        slmat = np.broadcast_to(slmat, (HD, NH_LOC * HD))
        in_maps.append({
            "hiddenT": hiddenT,
            "wqall": wqall,
            "bqk": np.ascontiguousarray(bqk.astype(np.float32)),
            "vbias": np.ascontiguousarray(vbias.astype(np.float32)),
            "biasca": np.ascontiguousarray(biasca),
            "negbbc": negbbc,
            "slmat": np.ascontiguousarray(
                slmat.astype(np.float32)).astype(bf16),
            "maskst": maskst,
            "wdr": wdr,
            "bdense": bdr,
        })
    return in_maps


def kernel(hidden_states, alibi, w_qkv, b_qkv, w_dense, b_dense):
    _ensure_axon_hooks()
    from concourse import bass_utils

    if "nc" not in _CACHE:
        _CACHE["nc"] = _build_nc()
    nc = _CACHE["nc"]
    in_maps = _prep_shards(hidden_states, alibi, w_qkv, b_qkv,
                           w_dense, b_dense)
    trace = bool(os.environ.get("BLOOM_TRACE"))
    res = bass_utils.run_bass_kernel_spmd(
        nc, in_maps, core_ids=list(range(NCORES)), trace=trace)
    kernel._last_results = res
    kernel._last_exec_ns = res.exec_time_ns
    outp = np.concatenate([res.results[c]["out"] for c in range(NCORES)],
                          axis=0)
    return outp.reshape(B, S, HID).astype(np.float32)
